# revision 24
# baseline (speedup 1.0000x reference)
"""Trainium2 Bass kernel for nn_BeAttentionGPT (single-head causal attention GPT block).

Computation per batch b (B=8, S=2048, H=1024):
    Q = x @ Wq.T + bq ; K = x @ Wk.T + bk ; V = x @ Wv.T + bv
    scores = Q @ K.T / sqrt(H), causal+pad masked (masked -> -1e9)
    attn = softmax(scores); out = attn @ V
Fully-padded query rows degenerate to a uniform average of all V rows.

Sharding: data-parallel over batch -- one batch per NeuronCore (8 cores).

Algebraic restructuring:
    Q.K^T = x A x^T + u.x_k + v.x_q + bq.bk   with A = Wq^T Wk, u = bq Wk.
    The per-q term and the constant cancel in the kernel's own row
    normalization and are dropped; u.x_k is folded host-side into the exp
    bias.  The device computes Z = A xc^T, S^T = Z contracted with xc^T,
    P = exp(S^T/sqrt(H) + bias), V0 = xc Wv^T (bv re-added on host), and
    out = normalized P^T V0 (row sums via a ones-column matmul).

Mask compaction: the pad mask invalidates ~half the positions; q and k
share it, so the host gathers the valid rows of x once into compacted
xc [SC=1152, H] (zero-padded).  Because the gather preserves order,
causality in compacted coordinates is STILL the triangular mask, so the
static causal trapezoid (skip strictly-upper tiles, min-cap the
diagonal 128x128 with a tril constant) survives compaction.  Z / V0 /
S^T / P.V0 all run on ~56% of the rows AND columns, and attention
keeps its ~2x causal saving on top.  Invalid queries are filled on the
host (mean-V fixup); a mask with more than SC valid positions falls
back to exact host math (never happens for ~50% masks).

All transposes/casts/packing are host-side numpy: the device receives
xc^T / A^T / Wv^T pre-cast to bf16 and pre-packed into single wide SBUF
tiles whose DMA is a handful of large fully-contiguous transfers (DMA
trigger instructions are ~0.6us each on the issuing engine, so fewer is
faster).  No PE transposes, no cast-DMA.
"""

import numpy as np
import ml_dtypes

B, S, H = 8, 2048, 1024
P = 128
SB = 512                 # column-group width
NH = H // P              # 8 h-chunks
SC = 1152                # compacted position capacity (9 chunks of 128)
NSC = SC // P            # 9 compacted chunks
SCALE = 1.0 / float(np.sqrt(np.float32(H)))
KBIAS = -30000.0         # dummy-key bias: exp(s/32 - 30000) == 0
CAP = -60000.0           # causal diag cap: exp(CAP/32 + anything) == 0

_CACHE = {}

# column groups over the 1152 compacted columns
CG = [(0, 512), (512, 512), (1024, 128)]


def _build_program():
    import concourse.bacc as bacc
    import concourse.tile as tile
    from concourse import mybir

    f32 = mybir.dt.float32
    bf16 = mybir.dt.bfloat16
    AF = mybir.ActivationFunctionType
    ALU = mybir.AluOpType

    nc = bacc.Bacc("TRN2", target_bir_lowering=False, debug=False)

    # ---- DRAM I/O ----
    # A^T packed cb-major: host layout [128p, 8cb, 8h, 128c]; the device
    # lhsT slice for (hp, h) is columns hp*H + h*P .. +P.
    ATp_d = nc.dram_tensor("ATp", [P, NH * H], bf16, kind="ExternalInput").ap()
    # xc^T packed chunk-major: [128p, 9s, 8h, 128c]; one contiguous 256KB
    # block per compacted 128-chunk.
    xcp_d = nc.dram_tensor("xcp", [P, NH * SC], bf16, kind="ExternalInput").ap()
    # Wv^T packed half-major: [128p, 2half, 8h, 512o]
    wvp_d = nc.dram_tensor("wvp", [P, NH * H], bf16, kind="ExternalInput").ap()
    ones_col_d = nc.dram_tensor("ones_col", [P, 1], bf16, kind="ExternalInput").ap()
    kbias_col_d = nc.dram_tensor("kbias_col", [P, NSC], f32, kind="ExternalInput").ap()
    tri_d = nc.dram_tensor("tri_cap", [P, P], f32, kind="ExternalInput").ap()
    # output in bf16: halves the output-DMA drain that forms the kernel
    # tail; adds ~0.4% RMS rounding, far inside the error budget
    out_d = nc.dram_tensor("out", [SC, H], bf16, kind="ExternalOutput").ap()

    GOFF = [NH * g0 for g0, gw in CG]  # flat column offset of each group

    with tile.TileContext(nc) as tc:
        from contextlib import ExitStack

        with ExitStack() as ctx:
            consts = ctx.enter_context(tc.tile_pool(name="consts", bufs=1))
            big = ctx.enter_context(tc.tile_pool(name="big", bufs=1))
            pt_pool = ctx.enter_context(tc.tile_pool(name="pt", bufs=1))
            PT_BUFS = {0: 4, 1: 8, 2: 9}  # live P tiles per column group
            out_pool = ctx.enter_context(tc.tile_pool(name="outp", bufs=4))
            small = ctx.enter_context(tc.tile_pool(name="small", bufs=4))
            psT = ctx.enter_context(tc.tile_pool(name="psT", bufs=2, space="PSUM"))
            psA = ctx.enter_context(tc.tile_pool(name="psA", bufs=4, space="PSUM"))

            # ---- constants ----
            ones_col = consts.tile([P, 1], bf16, tag="onesc")
            nc.sync.dma_start(out=ones_col, in_=ones_col_d)
            kbias_sb = consts.tile([P, NSC], f32, tag="kbias")
            nc.sync.dma_start(out=kbias_sb, in_=kbias_col_d)
            tri_sb = consts.tile([P, P], f32, tag="tri")
            nc.sync.dma_start(out=tri_sb, in_=tri_d)

            at = big.tile([P, NH * H], bf16, tag="at")      # [p, 8cb*8h*128c]
            xc = big.tile([P, NH * SC], bf16, tag="xc")     # [p, groups(8h*gw)]
            wv = big.tile([P, NH * H], bf16, tag="wv")      # [p, 8h*1024o]
            z = big.tile([P, NH * SC], bf16, tag="z")       # same layout as xc
            v = big.tile([P, NSC * H], bf16, tag="v")       # [p, 9s*1024o]

            def at_sl(hp, h):
                return at[:, hp * H + h * P:hp * H + (h + 1) * P]

            xc4 = xc.rearrange("p (s h c) -> p s h c", s=NSC, h=NH)

            def xc_rhs(gi, h, qoff):  # group gi columns qoff.. as 3D AP
                g0, gw = CG[gi]
                return xc4[:, (g0 + qoff) // P:(g0 + gw) // P, h, :]

            def xc_chunk(s, h):
                return xc4[:, s, h, :]

            def z_sl(gi, h, c0, c1):
                g0, gw = CG[gi]
                base = GOFF[gi] + h * gw
                return z[:, base + c0:base + c1]

            def wv_sl(h, half):  # half-major pack
                base = half * (NH * SB) + h * SB
                return wv[:, base:base + SB]

            # chunk s (0..8) -> (group, column offset within group)
            def chunk_loc(s):
                gi = 0 if s < 4 else (1 if s < 8 else 2)
                return gi, s * P - CG[gi][0]

            # ---- input DMA ----
            # Startup-critical transfers are split into ~256KB pieces so they
            # spread across parallel DMA queues (a single queue sustains only
            # ~100 GB/s); later transfers can be coarser.
            def dma_cols(eng, dst, src, c0, c1, pieces):
                step = (c1 - c0) // pieces
                for k in range(pieces):
                    a = c0 + k * step
                    b = c1 if k == pieces - 1 else a + step
                    eng.dma_start(out=dst[:, a:b], in_=src[:, a:b])

            # Z-first startup: gate is A^T block 0 (256KB) + xc chunks 0-3
            # (1MB), alternated across the two DMA-issuing engines in the
            # order Z consumes them (each engine's queue sustains only
            # ~130 GB/s, so need-order interleave is what keeps the PE fed).
            dma_cols(nc.sync, at, ATp_d, 0, H, 1)              # A^T cb 0
            for s in range(4):                                  # xc group 0
                eng = nc.scalar if s % 2 == 0 else nc.sync
                dma_cols(eng, xc, xcp_d, s * H, (s + 1) * H, 1)
            dma_cols(nc.sync, at, ATp_d, H, 4 * H, 2)          # A^T cb 1-3
            for s in range(4, 8):                               # xc group 1
                eng = nc.scalar if s % 2 == 0 else nc.sync
                dma_cols(eng, xc, xcp_d, s * H, (s + 1) * H, 1)
            dma_cols(nc.scalar, at, ATp_d, 4 * H, NH * H, 2)   # A^T cb 4-7
            dma_cols(nc.sync, xc, xcp_d, 8 * H, NSC * H, 1)     # xc group 2
            dma_cols(nc.sync, wv, wvp_d, 0, NH * SB, 2)        # wv half 0
            dma_cols(nc.scalar, wv, wvp_d, NH * SB, NH * H, 2)  # wv half 1

            evict_ctr = [0]

            def evict(dst, src):
                if evict_ctr[0] % 2 == 0:
                    nc.scalar.activation(dst, src, AF.Copy)
                else:
                    nc.vector.tensor_copy(dst, src)
                evict_ctr[0] += 1

            # ---- Z = A xc^T (sequential accumulation chains) ----
            for gi, (g0, gw) in enumerate(CG):
                for hp in range(NH):
                    ps = psA.tile([P, gw], f32, tag="psA", name="psA_t")
                    for h in range(NH):
                        nc.tensor.matmul(
                            ps,
                            lhsT=at_sl(hp, h),
                            rhs=xc_rhs(gi, h, 0),
                            start=(h == 0),
                            stop=(h == NH - 1),
                        )
                    evict(z_sl(gi, hp, 0, gw), ps)

            # ---- V0 projection first (smallest startup gate; no bias) ----
            for half in range(2):
                for s in range(NSC):
                    ps = psA.tile([P, SB], f32, tag="psA", name="psA_t")
                    for h in range(NH):
                        nc.tensor.matmul(
                            ps,
                            lhsT=xc_chunk(s, h),
                            rhs=wv_sl(h, half),
                            start=(h == 0),
                            stop=(h == NH - 1),
                        )
                    evict(v[:, s * H + half * SB:s * H + (half + 1) * SB], ps)

            # ---- scores: S^T[k~, q~] = sum_h Z[h, k~] xc[q~, h] ----
            # causal trapezoid in compacted coords: tile (i, gi) computed only
            # if chunk i is not strictly above the group's q-range; diagonal
            # 128x128 sub-block min-capped with the tril constant.
            pts = {}
            for i in range(NSC):  # compacted k-chunk
                ig, ioff = chunk_loc(i)
                for gi, (g0, gw) in enumerate(CG):
                    if i * P >= g0 + gw:
                        continue  # strictly above the causal diagonal
                    qoff = max(i * P - g0, 0)
                    ps = psA.tile([P, gw], f32, tag="psA", name="psA_t")
                    for h in range(NH):
                        nc.tensor.matmul(
                            ps[:, qoff:gw],
                            lhsT=z_sl(ig, h, ioff, ioff + P),
                            rhs=xc_rhs(gi, h, qoff),
                            start=(h == 0),
                            stop=(h == NH - 1),
                        )
                    if i * P >= g0:  # diagonal-crossing tile
                        nc.vector.tensor_tensor(
                            ps[:, qoff:qoff + P],
                            ps[:, qoff:qoff + P],
                            tri_sb,
                            ALU.min,
                        )
                    pt = pt_pool.tile([P, gw], bf16, tag=f"pt{gi}",
                                      bufs=PT_BUFS[gi], name="pt_t")
                    nc.scalar.activation(
                        pt[:, qoff:gw], ps[:, qoff:gw], AF.Exp,
                        bias=kbias_sb[:, i:i + 1],
                        scale=SCALE,
                    )
                    pts[(i, gi)] = pt

            # ---- out = normalized P^T V0, one 128-row q~ block at a time ----
            # three sequential accumulation chains (two output halves + row
            # sums) -- interleaving chains across PSUM banks measurably slows
            # the PE pipeline.
            for gi, (g0, gw) in enumerate(CG):
                for j in range(g0 // P, (g0 + gw) // P):  # compacted q-block
                    qo = j * P - g0
                    ops = psT.tile([P, H], f32, tag="psT", name="psO_t")
                    sps = psA.tile([P, 1], f32, tag="psA", name="psS_t")
                    for half in range(2):
                        for i in range(j + 1):
                            nc.tensor.matmul(
                                ops[:, half * SB:(half + 1) * SB],
                                lhsT=pts[(i, gi)][:, qo:qo + P],
                                rhs=v[:, i * H + half * SB:i * H + (half + 1) * SB],
                                start=(i == 0),
                                stop=(i == j),
                            )
                    for i in range(j + 1):
                        nc.tensor.matmul(
                            sps, lhsT=pts[(i, gi)][:, qo:qo + P], rhs=ones_col,
                            start=(i == 0), stop=(i == j),
                        )
                    rr = small.tile([P, 1], f32, tag="rr", name="rr_t")
                    nc.vector.reciprocal(rr, sps)
                    # split evictions + DMAs to shorten the serial tail; the
                    # final block gets quarter-splits and both DMA engines
                    last_block = (gi == len(CG) - 1) and (j == (g0 + gw) // P - 1)
                    pieces = 4 if last_block else 2
                    pw = H // pieces
                    for pc in range(pieces):
                        outsb = out_pool.tile([P, pw], bf16,
                                              tag=f"outp{pieces}", bufs=4,
                                              name="outsb_t")
                        if (j + pc) % 2 == 0:
                            nc.scalar.activation(
                                outsb, ops[:, pc * pw:(pc + 1) * pw],
                                AF.Copy, scale=rr,
                            )
                        else:
                            nc.vector.tensor_scalar_mul(
                                outsb, ops[:, pc * pw:(pc + 1) * pw], rr,
                            )
                        eng = nc.sync if (j + pc) % 2 == 0 else nc.scalar
                        eng.dma_start(
                            out=out_d[j * P:(j + 1) * P, pc * pw:(pc + 1) * pw],
                            in_=outsb,
                        )

    nc.compile()
    return nc


def _get_program():
    if "nc" not in _CACHE:
        _CACHE["nc"] = _build_program()
    return _CACHE["nc"]


def _host_reference(xb, mb, Wq, bq, Wk, bk, Wv, bv):
    """Exact (f64) per-batch fallback, mirrors the reference computation."""
    xb = xb.astype(np.float64)
    Q = xb @ Wq.astype(np.float64).T + bq.astype(np.float64)
    K = xb @ Wk.astype(np.float64).T + bk.astype(np.float64)
    V = xb @ Wv.astype(np.float64).T + bv.astype(np.float64)
    sc = Q @ K.T / np.sqrt(np.float64(H))
    keep = np.tril(np.ones((S, S), bool)) & (mb[None, :] & mb[:, None])
    sc = np.where(keep, sc, -1e9)
    sc -= sc.max(axis=1, keepdims=True)
    Pm = np.exp(sc)
    return ((Pm @ V) / Pm.sum(axis=1, keepdims=True)).astype(np.float32)


def _make_in_maps(x, attention_mask, Wq, bq, Wk, bk, Wv, bv):
    bf16 = ml_dtypes.bfloat16
    f32 = np.float32
    in_maps = []
    fallback = []
    valid_idx = []
    # A = Wq^T Wk  =>  A^T = Wk^T Wq; pack [128p, 8cb, 8h, 128c]
    AT = (Wk.astype(f32).T @ Wq.astype(f32)).astype(bf16)
    ATp = np.ascontiguousarray(
        AT.reshape(NH, P, NH, P).transpose(1, 2, 0, 3).reshape(P, NH * H))
    # Wv^T pack half-major [128p, 2half, 8h, 512o] so the V projection
    # can start after only the first 1MB half has landed
    WvT = Wv.astype(f32).T.astype(bf16)
    wvp = np.ascontiguousarray(
        WvT.reshape(NH, P, 2, SB).transpose(1, 2, 0, 3).reshape(P, NH * H))
    u = bq.astype(f32) @ Wk.astype(f32)  # [H]; per-k score bias u.x_k
    ones_col = np.ones((P, 1), dtype=bf16)
    ii = np.arange(P)
    tri_cap = np.where(
        ii[:, None] > ii[None, :], np.float32(CAP), np.float32(3.0e38)
    ).astype(f32)
    for b in range(B):
        mb = attention_mask[b].astype(bool)
        xb = x[b].astype(f32)
        idx = np.nonzero(mb)[0]
        if len(idx) > SC:
            fallback.append(b)
            idx = idx[:SC]
        nk = len(idx)
        valid_idx.append(idx)
        xc = np.zeros((SC, H), dtype=f32)
        xc[:nk] = xb[idx]
        kb = np.full(SC, KBIAS, dtype=f32)
        kb[:nk] = (xc[:nk] @ u) * np.float32(SCALE)
        # xc^T packed chunk-major [128p, 9s, 8h, 128c]: the V projection
        # for chunk s needs only one contiguous 256KB block
        xcT = xc.T.astype(bf16)
        xcp = np.ascontiguousarray(
            xcT.reshape(NH, P, NSC, P).transpose(1, 2, 0, 3).reshape(P, NSC * H))
        in_maps.append({
            "ATp": ATp, "wvp": wvp,
            "xcp": np.ascontiguousarray(xcp),
            "ones_col": ones_col,
            "kbias_col": np.ascontiguousarray(kb.reshape(NSC, P).T.astype(f32)),
            "tri_cap": tri_cap,
        })
    return in_maps, fallback, valid_idx


def run_spmd(x, attention_mask, Wq, bq, Wk, bk, Wv, bv, **spmd_kwargs):
    """Build (cached), run on 8 cores, return (stacked output, BassKernelResults)."""
    from concourse import bass_utils

    nc = _get_program()
    in_maps, fallback, valid_idx = _make_in_maps(
        x, attention_mask, Wq, bq, Wk, bk, Wv, bv)
    res = bass_utils.run_bass_kernel_spmd(
        nc, in_maps, core_ids=list(range(B)), **spmd_kwargs
    )
    bvf = bv.astype(np.float32)
    out = np.empty((B, S, H), dtype=np.float32)
    for b in range(B):
        dev = np.asarray(res.results[b]["out"], dtype=np.float32)
        idx = valid_idx[b]
        # scatter compacted rows back; bv was dropped from the device V
        # projection and attn rows sum to 1, so += bv here is exact.
        out[b][idx] = dev[:len(idx)] + bvf
        inv = ~attention_mask[b].astype(bool)
        if inv.any():
            # fully-padded query rows reduce to the uniform mean of all V
            # rows; mean(V) == mean(x) @ Wv.T + bv by linearity.
            mv = (x[b].astype(np.float64).mean(axis=0) @
                  Wv.astype(np.float64).T + bv.astype(np.float64))
            out[b][inv] = mv.astype(np.float32)
    for b in fallback:  # mask had > SC valid keys (never with ~50% masks)
        out[b] = _host_reference(x[b], attention_mask[b].astype(bool),
                                 Wq, bq, Wk, bk, Wv, bv)
    return out, res


def kernel(x, attention_mask, Wq, bq, Wk, bk, Wv, bv):
    x = np.asarray(x)
    attention_mask = np.asarray(attention_mask)
    Wq, bq = np.asarray(Wq), np.asarray(bq)
    Wk, bk = np.asarray(Wk), np.asarray(bk)
    Wv, bv = np.asarray(Wv), np.asarray(bv)
    out, _ = run_spmd(x, attention_mask, Wq, bq, Wk, bk, Wv, bv)
    return out
